# revision 1
# baseline (speedup 1.0000x reference)
"""Binarized 3x3 conv + bias + ReLU + eval-mode BatchNorm, Trainium2 Bass kernel.

Problem: x[16,64,256,256] f32, w[64,64,3,3], per-channel b/gamma/beta/mean/var.
  y = BN(relu(conv(sign(x), sign(w)) + b))  (eval-mode BN = per-channel affine)

Strategy (8 NeuronCores, data-parallel over batch):
  - 2 images per core; image A on SBUF partitions 0-63 (channels), image B on 64-127.
  - Binarize on-chip as t = (x >= 0) in {1,0} bf16 (one DVE is_ge op); spatial
    padding uses 0.5 so that the identity  conv_pm = 2*conv_t - S  holds exactly
    (S[co] = sum of sign(w) over taps; pads contribute 2*0.5-1 = 0).
  - 3x3 conv = 9 accumulating matmuls per PSUM tile (K=Cin=64, M=Cout=64),
    using 64x64 PE array tiling: 4 quadrants = (imgA,imgB) x (left,right 128-col
    half) run concurrently -> full 128x128 array utilization.
  - Post: ScalarE relu(2*psum + (b-S)) then VectorE y = t*inv + c, both with
    per-partition vectors; one 128-partition DMA per image per 32-row block.
  - Weights/BN vectors are tiny and prepped on host: lhsT bf16 [128, 9*64]
    (sign(w) transposed to [ci, tap, co], replicated to both partition halves).
"""

import numpy as np
import ml_dtypes

import concourse.bass as bass  # noqa: F401  (AP types ride along)
import concourse.mybir as mybir
import concourse.tile as tile
from concourse import bacc
from concourse.bass_utils import run_bass_kernel_spmd

N_CORES = 8
IMGS_PER_CORE = 2
C = 64
H = 256
W = 256
RB = 32              # output rows per block
NBLK = H // RB       # 8
ROWS_IN = RB + 4     # input-buffer rows (1-row halo each side + even-count slack)
WP = W + 4           # padded row width in xb; data at col offset 2
BN_EPS = 1e-5
DT = mybir.dt

_PROGRAM = None


def _build():
    nc = bacc.Bacc(
        "TRN2",
        target_bir_lowering=False,
        debug=False,
        enable_asserts=False,
    )
    x = nc.dram_tensor("x", [IMGS_PER_CORE, C, H, W], DT.float32, kind="ExternalInput")
    wT = nc.dram_tensor("wT", [128, 9 * 64], DT.bfloat16, kind="ExternalInput")
    bvec = nc.dram_tensor("bvec", [128, 1], DT.float32, kind="ExternalInput")
    ivec = nc.dram_tensor("ivec", [128, 1], DT.float32, kind="ExternalInput")
    cvec = nc.dram_tensor("cvec", [128, 1], DT.float32, kind="ExternalInput")
    y = nc.dram_tensor("y", [IMGS_PER_CORE, C, H, W], DT.float32, kind="ExternalOutput")

    x_flat = x.ap().rearrange("n c h w -> (n c) (h w)")   # [128, 65536] flat
    y_m = y.ap().rearrange("n c h w -> (n c) (h w)")      # [128, 65536] flat
    y_n0 = y.ap()[0].rearrange("c h w -> c (h w)")        # [64, 65536]
    y_n1 = y.ap()[1].rearrange("c h w -> c (h w)")        # [64, 65536]

    with tile.TileContext(nc) as tc:
        with (
            tc.tile_pool(name="consts", bufs=1) as cpool,
            tc.tile_pool(name="xin", bufs=2) as xpool,
            tc.tile_pool(name="xbp", bufs=2) as xbpool,
            tc.tile_pool(name="tsb", bufs=4) as tpool,
            tc.tile_pool(name="yout", bufs=2) as ypool,
            tc.tile_pool(name="psum", bufs=2, space="PSUM") as ppool,
        ):
            wt = cpool.tile([128, 9 * 64], DT.bfloat16, tag="wt")
            bv = cpool.tile([128, 1], DT.float32, tag="bv")
            iv = cpool.tile([128, 1], DT.float32, tag="iv")
            cv = cpool.tile([128, 1], DT.float32, tag="cv")

            def load_consts():
                nc.sync.dma_start(wt[:], wT.ap())
                nc.sync.dma_start(bv[:], bvec.ap())
                nc.sync.dma_start(iv[:], ivec.ap())
                nc.sync.dma_start(cv[:], cvec.ap())

            def load_block(blk):
                """DMA block's input rows (always an even 34-row span, for DVE
                2x mode) and binarize into the padded bf16 tile."""
                r0 = blk * RB
                ndma = RB + 2
                if blk == 0:
                    lo_c, off = 0, 1          # buffer row k = x row k-1
                elif blk == NBLK - 1:
                    lo_c, off = r0 - 2, 0     # buffer row k = x row r0-2+k
                else:
                    lo_c, off = r0 - 1, 0     # buffer row k = x row r0-1+k

                xin = xpool.tile([128, ROWS_IN * W], DT.float32, tag="xin")
                xin_v = xin[:].rearrange("p (r c) -> p r c", c=W)
                xb = xbpool.tile([128, ROWS_IN * WP], DT.bfloat16, tag="xb")
                xb_v = xb[:].rearrange("p (r c) -> p r c", c=WP)
                # two chunks (even row counts for DVE 2x) so matmuls of the top
                # half can start while the bottom half still transfers
                for a, b in ((0, ndma),):
                    nc.sync.dma_start(
                        xin[:, (off + a) * W : (off + b) * W],
                        x_flat[:, (lo_c + a) * W : (lo_c + b) * W],
                    )
                    nc.vector.tensor_scalar(
                        xb_v[:, off + a : off + b, 2 : 2 + W],
                        xin_v[:, off + a : off + b, :],
                        0.0,
                        None,
                        op0=mybir.AluOpType.is_ge,
                    )
                nc.vector.memset(xb_v[:, :, 0:2], 0.5)
                nc.vector.memset(xb_v[:, :, 2 + W : WP], 0.5)
                if blk == 0:
                    nc.vector.memset(xb_v[:, 0:1, :], 0.5)
                if blk == NBLK - 1:
                    nc.vector.memset(xb_v[:, ndma : ndma + 1, :], 0.5)
                return xb_v

            def compute_block(blk, xb_v):
                """Matmuls + post-ops + output DMAs for a loaded block."""
                r0 = blk * RB
                hb = RB // 2  # rows per half-block (16)
                row_shift = 1 if blk == NBLK - 1 else 0
                # PSUM bank T = [imgA-top | imgB-top] (partition = n*64+c);
                # bank B = [imgB-bot | imgA-bot] (image-reversed).
                # Quadrants: A-T=(0,0)  B-T=(64,64)  B-B=(64,0)  A-B=(0,64)
                # Output staging is full-width rows -> contiguous DMA descriptors.
                yt_ = ypool.tile([128, hb * W], DT.float32, tag="ytop")
                yb_ = ypool.tile([128, hb * W], DT.float32, tag="ybot")
                for it2 in range(hb // 4):          # 4 output rows per super-tile
                    # super-tiles spanning 2 PSUM banks; each matmul stays in one
                    ps_t = ppool.tile([128, 1024], DT.float32, tag="pst")
                    ps_b = ppool.tile([128, 1024], DT.float32, tag="psb")
                    for sub in range(2):            # 2 rows per matmul set
                        it = 2 * it2 + sub
                        c0 = sub * 512
                        for t in range(9):
                            dy, dx = divmod(t, 3)
                            first, last = (t == 0), (t == 8)
                            rt = 2 * it + dy + row_shift          # top-half rows
                            rb_ = hb + 2 * it + dy + row_shift    # bottom-half rows
                            cs = 1 + dx
                            quads = (
                                (ps_t, 0, 0, rt),      # A-top -> psT[0:64]
                                (ps_t, 64, 64, rt),    # B-top -> psT[64:128]
                                (ps_b, 64, 0, rb_),    # B-bot -> psB[0:64]
                                (ps_b, 0, 64, rb_),    # A-bot -> psB[64:128]
                            )
                            for ps, xp0, op0_, rlo in quads:
                                wslc = wt[xp0 : xp0 + 64, t * 64 : (t + 1) * 64]
                                rhs = xb_v[xp0 : xp0 + 64, rlo : rlo + 2, cs : cs + W]
                                nc.tensor.matmul(
                                    ps[op0_ : op0_ + 64, c0 : c0 + 512],
                                    wslc,
                                    rhs,
                                    start=first,
                                    stop=last,
                                )
                    for ps, yst in ((ps_t, yt_), (ps_b, yb_)):
                        tsb = tpool.tile([128, 1024], DT.float32, tag="tsb")
                        nc.scalar.activation(
                            tsb[:],
                            ps[:],
                            mybir.ActivationFunctionType.Relu,
                            bias=bv[:],
                            scale=2.0,
                        )
                        nc.vector.tensor_scalar(
                            yst[:, it2 * 1024 : (it2 + 1) * 1024],
                            tsb[:],
                            iv[:],
                            cv[:],
                            op0=mybir.AluOpType.mult,
                            op1=mybir.AluOpType.add,
                        )
                rb0 = r0 + hb
                nsplit = 2 if blk == NBLK - 1 else 1
                step = hb // nsplit
                for s0 in range(0, hb, step):
                    nc.sync.dma_start(
                        y_m[:, (r0 + s0) * W : (r0 + s0 + step) * W],
                        yt_[:, s0 * W : (s0 + step) * W],
                    )
                    nc.gpsimd.dma_start(
                        y_n1[:, (rb0 + s0) * W : (rb0 + s0 + step) * W],
                        yb_[0:64, s0 * W : (s0 + step) * W],
                    )
                    nc.gpsimd.dma_start(
                        y_n0[:, (rb0 + s0) * W : (rb0 + s0 + step) * W],
                        yb_[64:128, s0 * W : (s0 + step) * W],
                    )

            # software pipeline: queue block i+1's input DMA before block i's
            # output DMAs so the FIFO DMA ring never head-of-line blocks
            pending = None
            for blk in range(NBLK):
                xb_v = load_block(blk)
                if blk == 0:
                    load_consts()
                if pending is not None:
                    compute_block(pending[0], pending[1])
                pending = (blk, xb_v)
            compute_block(pending[0], pending[1])
    nc.compile()
    return nc


def _get_program():
    global _PROGRAM
    if _PROGRAM is None:
        _PROGRAM = _build()
    return _PROGRAM


def _prep_params(w, b, gamma, beta, running_mean, running_var):
    wb = np.where(w >= 0, 1.0, -1.0).astype(np.float32)          # [co, ci, ky, kx]
    wt = np.ascontiguousarray(wb.transpose(1, 2, 3, 0))          # [ci, ky, kx, co]
    wt = wt.reshape(C, 9 * C).astype(ml_dtypes.bfloat16)
    wt2 = np.ascontiguousarray(np.concatenate([wt, wt], axis=0))  # [128, 576]
    s = wb.sum(axis=(1, 2, 3)).astype(np.float32)
    inv = (gamma.astype(np.float32) / np.sqrt(running_var.astype(np.float32) + BN_EPS)).astype(np.float32)
    cc = (beta.astype(np.float32) - running_mean.astype(np.float32) * inv).astype(np.float32)
    bp = (b.astype(np.float32) - s).astype(np.float32)

    def rep(v):
        return np.ascontiguousarray(np.tile(v.astype(np.float32), 2).reshape(128, 1))

    return wt2, rep(bp), rep(inv), rep(cc)


def run(x, w, b, gamma, beta, running_mean, running_var, trace=False):
    nc = _get_program()
    wt2, bp, inv, cc = _prep_params(w, b, gamma, beta, running_mean, running_var)
    x = np.asarray(x, dtype=np.float32)
    in_maps = []
    for i in range(N_CORES):
        in_maps.append(
            {
                "x": np.ascontiguousarray(x[IMGS_PER_CORE * i : IMGS_PER_CORE * (i + 1)]),
                "wT": wt2,
                "bvec": bp,
                "ivec": inv,
                "cvec": cc,
            }
        )
    res = run_bass_kernel_spmd(nc, in_maps, list(range(N_CORES)), trace=trace)
    y = np.concatenate([res.results[i]["y"] for i in range(N_CORES)], axis=0)
    return y, res


def kernel(x, w, b, gamma, beta, running_mean, running_var):
    y, _ = run(x, w, b, gamma, beta, running_mean, running_var)
    return y

